# revision 1
# baseline (speedup 1.0000x reference)
# Trainium2 Bass kernel for nn_CrossAttentionLayer (linear attention with
# elu+1 feature map).
#
# Math (per batch n):
#   q = guidance @ Wq.T + bq ; k = x @ Wk.T + bk ; v = x @ Wv.T + bv
#   Q = elu(q)+1 ; K = elu(k)+1          (per head h, head dim D=64)
#   KV_h = K_h^T @ (v_h/S);  Z = 1/(Q_h . sum_s K_h + eps)
#   out_h = (Q_h @ KV_h) * Z * S         (the /S and *S cancel exactly)
#
# Sharding: 8 cores = batch(4) x guidance-halves(2). Each core recomputes
# K/V/KV/Ksum for its batch over the full source sequence S=4096 (dup x2),
# and the Q side for its 2048 guidance rows.
#
# On-chip dataflow (all matmuls in float32r: full-rate PE fp32):
#  phase 1 (per 128-row s-tile of x):
#    PE-transpose x-tile -> xT ; k/v projections token-major with xT as
#    stationary (bias added via a K=1 matmul against a ones row);
#    K = elu(k)+1 computed as max(relu(k+1), min(exp(k), 1));
#    KV accumulated in PSUM via 4 two-head matmuls (N=256); Ksum via a
#    ones-column matmul (N=512).
#  phase 2 (per 512-row l-chunk of guidance):
#    PE-transpose g -> gT ; q projection FEATURE-major (weights stationary,
#    per-partition bias via the activation); denominators via block-diagonal
#    Ksum matrix; out = (Q @ blockdiag(KV)) scaled by Z broadcast.

import sys

import numpy as np

if "/opt/trn_rl_repo" not in sys.path:
    sys.path.insert(0, "/opt/trn_rl_repo")

import concourse.bacc as bacc
import concourse.mybir as mybir
import concourse.tile as tile
from concourse import bass_utils
from concourse.masks import make_identity

P = 128
S = 4096
LC = 2048  # guidance rows per core
C = 512
H = 8
D = 64
NCT = C // P  # 4 column tiles
NST = S // P  # 32 s-tiles
EPS = 1e-6
SPLIT_S = True  # s-split across core pairs + AllReduce of partial KV/Ksum
SKIP_CC = False  # timing experiment: skip the collective (WRONG results)

F32 = mybir.dt.float32
F32R = mybir.dt.float32r

Exp = mybir.ActivationFunctionType.Exp
Relu = mybir.ActivationFunctionType.Relu


def _build_nc(reps=1, with_bias=False, split_s=None):
    if split_s is None:
        split_s = SPLIT_S
    nc = bacc.Bacc(
        "TRN2",
        target_bir_lowering=False,
        debug=False,
        enable_asserts=False,
        num_devices=8,
    )
    xs = S // 2 if split_s else S
    xb = nc.dram_tensor("xb", [xs, C], F32, kind="ExternalInput").ap()
    gb = nc.dram_tensor("gb", [LC, C], F32, kind="ExternalInput").ap()
    wkt = nc.dram_tensor("wkt", [C, C], F32, kind="ExternalInput").ap()
    wvt = nc.dram_tensor("wvt", [C, C], F32, kind="ExternalInput").ap()
    wqt = nc.dram_tensor("wqt", [C, C], F32, kind="ExternalInput").ap()
    bk = nc.dram_tensor("bk", [C], F32, kind="ExternalInput").ap()
    bv = nc.dram_tensor("bv", [C], F32, kind="ExternalInput").ap()
    bq = nc.dram_tensor("bq", [C], F32, kind="ExternalInput").ap()
    outb = nc.dram_tensor("outb", [LC, C], F32, kind="ExternalOutput").ap()

    with tile.TileContext(nc) as tc:
        for rep in range(reps):
            _emit(nc, tc, xb, gb, wkt, wvt, wqt, bk, bv, bq, outb, rep=rep,
                  with_bias=with_bias, split_s=split_s)

    nc.compile()
    return nc


def _emit(nc, tc, xb, gb, wkt, wvt, wqt, bk, bv, bq, outb, rep=0,
          with_bias=False, split_s=False):
    mm = nc.tensor.matmul
    nst = NST // 2 if split_s else NST
    with (
        tc.tile_pool(name=f"persist{rep}", bufs=1) as pp,
        tc.tile_pool(name=f"dram{rep}", bufs=1, space="DRAM") as dp,
    ):
        # --- constants / weights resident in SBUF ---
        # fp32r matmul operands must be produced by DVE/ACT compute ops (the
        # verifier requires an explicit rounding producer), so DMA/memset
        # results are staged in fp32 and copied into fp32r tiles on DVE.
        wk_sb = pp.tile([P, NCT, C], F32R)
        wv_sb = pp.tile([P, NCT, C], F32R)
        wq_sb = pp.tile([P, NCT, C], F32R)
        bk_row = pp.tile([1, C], F32R)
        bv_row = pp.tile([1, C], F32R)
        ones_row = pp.tile([1, P], F32R)
        ones_colr = pp.tile([P, 1], F32R)
        ident = pp.tile([P, P], F32)
        make_identity(nc, ident)
        if True:
            ip = pp
            wk_st = ip.tile([P, NCT, C], F32, name="wk_st")
            wv_st = ip.tile([P, NCT, C], F32, name="wv_st")
            wq_st = ip.tile([P, NCT, C], F32, name="wq_st")
            nc.gpsimd.dma_start(wk_st, wkt.rearrange("(t p) n -> p t n", p=P))
            nc.gpsimd.dma_start(wv_st, wvt.rearrange("(t p) n -> p t n", p=P))
            nc.gpsimd.dma_start(wq_st, wqt.rearrange("(t p) n -> p t n", p=P))
            nc.vector.tensor_copy(wk_sb, wk_st)
            nc.vector.tensor_copy(wv_sb, wv_st)
            nc.vector.tensor_copy(wq_sb, wq_st)
            bk_st = ip.tile([1, C], F32, name="bk_st")
            bv_st = ip.tile([1, C], F32, name="bv_st")
            nc.gpsimd.dma_start(bk_st, bk.rearrange("(a c) -> a c", a=1))
            nc.gpsimd.dma_start(bv_st, bv.rearrange("(a c) -> a c", a=1))
            nc.vector.tensor_copy(bk_row, bk_st)
            nc.vector.tensor_copy(bv_row, bv_st)
            ones_st = ip.tile([1, P], F32, name="ones_st")
            nc.vector.memset(ones_st, 1.0)
            nc.vector.tensor_copy(ones_row, ones_st)
            onescol_st = ip.tile([P, 1], F32, name="onescol_st")
            nc.vector.memset(onescol_st, 1.0)
            nc.vector.tensor_copy(ones_colr, onescol_st)
        bqT = pp.tile([P, NCT], F32)
        nc.sync.dma_start(bqT, bq.rearrange("(t p) -> p t", p=P))
        bqT1 = pp.tile([P, NCT], F32)
        nc.vector.tensor_scalar_add(bqT1, bqT, 1.0)

        zero_col = pp.tile([P, 1], F32)
        nc.vector.memset(zero_col, 0.0)
        ones_col = pp.tile([P, 1], F32)
        nc.vector.memset(ones_col, 1.0)
        onezero = pp.tile([P, 2], F32)
        nc.vector.memset(onezero[:, 0:1], 1.0)
        nc.vector.memset(onezero[:, 1:2], 0.0)

        # blockdiag(KV_h) as [cin_part, cin_tile, C] and blockdiag Ksum
        kvbd = pp.tile([P, NCT, C], F32R)
        nc.vector.tensor_copy(kvbd, zero_col[:, :, None].to_broadcast([P, NCT, C]))
        ksbd = pp.tile([P, NCT, H], F32R)
        nc.vector.tensor_copy(ksbd, zero_col[:, :, None].to_broadcast([P, NCT, H]))
        ksumT = pp.tile([P, NCT], F32)

        # ---------------- phase 1: x -> K,V -> KV, Ksum ----------------
        # Each KV accumulation group owns a full PSUM bank (start=True zeroes
        # the whole 2KB zero region). The V operand carries an extra ones
        # column so column 256 of each KV psum accumulates Ksum directly in
        # feature-major layout.
        with (
            tc.tile_pool(name=f"p1_{rep}", bufs=3) as p1,
            tc.tile_pool(name=f"p1ps_{rep}", bufs=3, space="PSUM") as p1ps,
            tc.tile_pool(name=f"tps_{rep}", bufs=2, space="PSUM") as tps,
            tc.tile_pool(name=f"accps_{rep}", bufs=1, space="PSUM") as accps,
        ):
            kv_ps = [
                accps.tile([P, 2, 256], F32, tag=f"kv{b}", name=f"kv_ps{b}")
                for b in range(2)
            ]
            ksum_ps = accps.tile([1, C], F32, name="ksum_ps")

            def kv_mms(kv, first, last):
                # KV accumulation: two K-heads vs four V-heads per matmul.
                # Two accumulation groups share each PSUM bank: only the
                # first matmul into a bank uses start=True (it zeroes the
                # whole 2KB zero region), only the last uses stop=True.
                k_sb, v_ext = kv
                for hh in range(4):
                    mm(kv_ps[hh // 2][:, hh % 2, :],
                       k_sb[:, hh * P : (hh + 1) * P],
                       v_ext[:, hh // 2, :],
                       start=(first and hh % 2 == 0),
                       stop=(last and hh % 2 == 1))
                mm(ksum_ps, ones_colr, k_sb, start=first, stop=last)

            def consume(stage, first, last):
                # V split into two 4-head halves, each with a ones column
                # (and a zero pad column: fp32r matmuls need an even free dim)
                pk, pv = stage
                # K = elu(k)+1 = max(relu(k+1), min(exp(k), 1))
                e_sb = p1.tile([P, C], F32, tag="e")
                nc.scalar.activation(e_sb, pk, Exp)
                u_sb = p1.tile([P, C], F32, tag="u")
                nc.scalar.activation(u_sb, pk, Relu, bias=1.0)
                nc.vector.tensor_scalar_min(e_sb, e_sb, 1.0)
                k_sb = p1.tile([P, C], F32R, tag="k")
                nc.vector.tensor_tensor(k_sb, e_sb, u_sb, mybir.AluOpType.max)
                v_ext = p1.tile([P, 2, 256], F32R, tag="v")
                nc.scalar.copy(v_ext[:, 0, :], pv[:, 0:256])
                nc.scalar.copy(v_ext[:, 1, :], pv[:, 256:512])
                kv_mms((k_sb, v_ext), first, last)

            prev_stage = None
            for st in range(nst):
                xt = p1.tile([P, C], F32, tag="xt")
                nc.sync.dma_start(xt, xb[st * P : (st + 1) * P, :])
                xT = p1.tile([P, NCT, P], F32R, tag="xT")
                pt = tps.tile([P, NCT, P], F32, tag="tp")
                for ci in range(NCT):
                    mm(pt[:, ci, :], xt[:, ci * P : (ci + 1) * P], ident,
                       is_transpose=True,
                       start=(ci == 0), stop=(ci == NCT - 1))
                nc.vector.tensor_copy(xT, pt)
                # k projection (token-major): psum[s,cout]
                pk = p1ps.tile([P, C], F32, tag="proj")
                if with_bias:
                    mm(pk, ones_row, bk_row, start=True, stop=False)
                for ci in range(NCT):
                    mm(pk, xT[:, ci, :], wk_sb[:, ci, :],
                       start=(ci == 0 and not with_bias),
                       stop=(ci == NCT - 1))
                # v projection
                pv = p1ps.tile([P, C], F32, tag="proj")
                if with_bias:
                    mm(pv, ones_row, bv_row, start=True, stop=False)
                for ci in range(NCT):
                    mm(pv, xT[:, ci, :], wv_sb[:, ci, :],
                       start=(ci == 0 and not with_bias),
                       stop=(ci == NCT - 1))
                # software pipeline: consume the PREVIOUS iteration's psum
                # (elu + KV matmuls) so no engine queue ever heads-of-line
                # blocks this iteration's PE feed chain
                if prev_stage is not None:
                    consume(prev_stage, st == 1, False)
                prev_stage = (pk, pv)
            consume(prev_stage, False, True)

            # Ksum [1, C] -> feature-major [128, 4] via a DRAM round-trip
            ksum_row = pp.tile([1, C], F32)
            nc.vector.tensor_copy(ksum_row, ksum_ps)
            scratch = dp.tile([1, C], F32, name="scratch")
            nc.sync.dma_start(scratch, ksum_row)
            nc.sync.dma_start(
                ksumT, scratch.rearrange("a (t p) -> (a p) t", p=P)
            )
            if split_s:
                # pack partial KV banks + KsumT, AllReduce across the core
                # pair sharing this batch, then unpack the full sums
                stg = pp.tile([P, 1028], F32)
                nc.vector.tensor_copy(
                    stg[:, 0:512].rearrange("p (a v) -> p a v", a=2),
                    kv_ps[0],
                )
                nc.vector.tensor_copy(
                    stg[:, 512:1024].rearrange("p (a v) -> p a v", a=2),
                    kv_ps[1],
                )
                nc.vector.tensor_copy(stg[:, 1024:1028], ksumT)
                ccin = nc.dram_tensor(
                    f"ccin{rep}", [P, 1028], F32
                ).ap()
                ccout = nc.dram_tensor(
                    f"ccout{rep}", [P, 1028], F32
                ).ap()
                nc.sync.dma_start(ccin, stg)
                if not SKIP_CC:
                    nc.gpsimd.collective_compute(
                        "AllReduce",
                        mybir.AluOpType.add,
                        replica_groups=[[0, 1], [2, 3], [4, 5], [6, 7]],
                        ins=[ccin],
                        outs=[ccout],
                    )
                stg2 = pp.tile([P, 1028], F32)
                nc.sync.dma_start(stg2, ccout if not SKIP_CC else ccin)
                kv_src = [
                    stg2[:, 0:512].rearrange("p (a v) -> p a v", a=2),
                    stg2[:, 512:1024].rearrange("p (a v) -> p a v", a=2),
                ]
                ksum_src = stg2[:, 1024:1028]
            else:
                kv_src = kv_ps
                ksum_src = ksumT
            # extract per-head KV blocks into blockdiag layout
            for h in range(H):
                hh = h // 2
                par = h % 2
                vcol = (h % 4) * D
                nc.vector.tensor_copy(
                    kvbd[par * D : (par + 1) * D, hh, h * D : (h + 1) * D],
                    kv_src[hh // 2][par * D : (par + 1) * D, hh % 2,
                                    vcol : vcol + D],
                )
            # blockdiag Ksum [cin_part, cin_tile, H]
            for h in range(H):
                par = h % 2
                ct = h // 2
                nc.vector.tensor_copy(
                    ksbd[par * D : (par + 1) * D, ct, h : h + 1],
                    ksum_src[par * D : (par + 1) * D, ct : ct + 1],
                )

        # ---------------- phase 2: guidance -> Q -> out ----------------
        with (
            tc.tile_pool(name=f"p2_{rep}", bufs=2) as p2,
            tc.tile_pool(name=f"gtp_{rep}", bufs=4) as gtp,
            tc.tile_pool(name=f"p2ps_{rep}", bufs=3, space="PSUM") as p2ps,
            tc.tile_pool(name=f"pops_{rep}", bufs=2, space="PSUM") as pops,
            tc.tile_pool(name=f"tps2_{rep}", bufs=1, space="PSUM") as tps2,
            tc.tile_pool(name=f"dps_{rep}", bufs=2, space="PSUM") as dps,
        ):
            def q_tail(qT, lc):
                # per 128-row l-tile: denominators, then output
                for lt in range(4):
                    lsl = slice(lt * P, (lt + 1) * P)
                    pd = dps.tile([P, H], F32, tag="pd")
                    for ct in range(NCT):
                        mm(pd, qT[:, ct, lsl], ksbd[:, ct, :],
                           start=(ct == 0), stop=(ct == NCT - 1))
                    zl = p2.tile([P, H], F32, tag="zl")
                    nc.vector.tensor_scalar_add(zl, pd, EPS)
                    nc.vector.reciprocal(zl, zl)
                    po = pops.tile([P, C], F32, tag="po")
                    for ct in range(NCT):
                        mm(po, qT[:, ct, lsl], kvbd[:, ct, :],
                           start=(ct == 0), stop=(ct == NCT - 1))
                    osb = p2.tile([P, C], F32, tag="osb")
                    nc.vector.tensor_tensor(
                        osb.rearrange("p (h v) -> p h v", h=H),
                        po.rearrange("p (h v) -> p h v", h=H),
                        zl[:, :, None].to_broadcast([P, H, D]),
                        mybir.AluOpType.mult,
                    )
                    nc.sync.dma_start(
                        outb[(lc * 4 + lt) * P : (lc * 4 + lt + 1) * P, :], osb
                    )

            prev_q = None
            for lc in range(LC // C):
                gT = p2.tile([P, NCT, C], F32R, tag="gT")
                for lt in range(4):
                    gt = gtp.tile([P, C], F32, tag="gt")
                    nc.sync.dma_start(
                        gt, gb[(lc * 4 + lt) * P : (lc * 4 + lt + 1) * P, :]
                    )
                    pt = tps2.tile([P, NCT, P], F32, tag="tp2")
                    for ci in range(NCT):
                        mm(pt[:, ci, :], gt[:, ci * P : (ci + 1) * P], ident,
                           is_transpose=True,
                           start=(ci == 0), stop=(ci == NCT - 1))
                    nc.vector.tensor_copy(gT[:, :, lt * P : (lt + 1) * P], pt)
                # q projection, feature-major: psum[cout, l]
                qT = p2.tile([P, NCT, C], F32R, tag="qT")
                pqs = []
                for ct in range(NCT):
                    pq = p2ps.tile([P, C], F32, tag="pq")
                    for ci in range(NCT):
                        mm(pq, wq_sb[:, ci, ct * P : (ct + 1) * P],
                           gT[:, ci, :], start=(ci == 0), stop=(ci == NCT - 1))
                    pqs.append(pq)
                # previous chunk's tail before this chunk's elu, so the tail
                # DVE/PE work isn't queued behind ACT-dependent elu ops
                if prev_q is not None:
                    q_tail(prev_q, lc - 1)
                for ct in range(NCT):
                    pq = pqs[ct]
                    e2 = p2.tile([P, C], F32, tag="e2")
                    nc.scalar.activation(e2, pq, Exp, bias=bqT[:, ct : ct + 1])
                    u2 = p2.tile([P, C], F32, tag="u2")
                    nc.scalar.activation(u2, pq, Relu, bias=bqT1[:, ct : ct + 1])
                    nc.vector.tensor_scalar_min(e2, e2, 1.0)
                    nc.vector.tensor_tensor(
                        qT[:, ct, :], e2, u2, mybir.AluOpType.max
                    )
                prev_q = qT
            q_tail(prev_q, LC // C - 1)


_CACHE = {}


def _get_nc(reps=1, with_bias=False):
    key = ("nc", reps, with_bias, SPLIT_S, SKIP_CC)
    if key not in _CACHE:
        _CACHE[key] = _build_nc(reps, with_bias, SPLIT_S)
    return _CACHE[key]


def _make_runner(nc):
    """Build a reusable jitted SPMD runner for `nc` (mirrors
    bass2jax.run_bass_via_pjrt's multi-core branch, but caches the jit so
    repeated calls don't re-lower/re-compile)."""
    import jax
    from jax.sharding import Mesh, PartitionSpec
    from jax.experimental.shard_map import shard_map

    import concourse.mybir as mb
    from concourse import bass2jax

    bass2jax.install_neuronx_cc_hook()

    n_cores = 8
    partition_name = (
        nc.partition_id_tensor.name if nc.partition_id_tensor else None
    )
    in_names, out_names, out_avals, zero_shapes = [], [], [], []
    for alloc in nc.m.functions[0].allocations:
        if not isinstance(alloc, mb.MemoryLocationSet):
            continue
        name = alloc.memorylocations[0].name
        if alloc.kind == "ExternalInput":
            if name != partition_name:
                in_names.append(name)
        elif alloc.kind == "ExternalOutput":
            shape = tuple(alloc.tensor_shape)
            dtype = mb.dt.np(alloc.dtype)
            out_names.append(name)
            out_avals.append(jax.core.ShapedArray(shape, dtype))
            zero_shapes.append((shape, dtype))
    n_params = len(in_names)
    n_outs = len(out_names)
    all_names = in_names + out_names
    if partition_name is not None:
        all_names.append(partition_name)
    donate = tuple(range(n_params, n_params + n_outs))

    def _body(*args):
        operands = list(args)
        if partition_name is not None:
            operands.append(bass2jax.partition_id_tensor())
        outs = bass2jax._bass_exec_p.bind(
            *operands,
            out_avals=tuple(out_avals),
            in_names=tuple(all_names),
            out_names=tuple(out_names),
            lowering_input_output_aliases=(),
            sim_require_finite=True,
            sim_require_nnan=True,
            nc=nc,
        )
        return tuple(outs)

    devices = jax.devices()[:n_cores]
    mesh = Mesh(np.asarray(devices), ("core",))
    in_specs = (PartitionSpec("core"),) * (n_params + n_outs)
    out_specs = (PartitionSpec("core"),) * n_outs
    sharded = jax.jit(
        shard_map(
            _body, mesh=mesh, in_specs=in_specs, out_specs=out_specs,
            check_rep=False,
        ),
        donate_argnums=donate,
        keep_unused=True,
    )

    def _zeros():
        return [
            np.zeros((n_cores * sh[0], *sh[1:]), dt) for sh, dt in zero_shapes
        ]

    def runner(concat_in):
        out_arrs = sharded(*concat_in, *_zeros())
        return [
            {
                name: np.asarray(out_arrs[i]).reshape(
                    n_cores, *out_avals[i].shape
                )[c]
                for i, name in enumerate(out_names)
            }
            for c in range(n_cores)
        ]

    def concat(maps):
        return [
            np.concatenate([np.asarray(m[name]) for m in maps], axis=0)
            for name in in_names
        ]

    def timed(concat_in, n=10, warmup=2):
        """Time `n` executions with device-resident inputs and on-device
        donated zero outputs, so per-call host traffic is ~zero."""
        import time as _time
        import jax.numpy as jnp
        from jax.sharding import NamedSharding

        sh = NamedSharding(mesh, PartitionSpec("core"))
        dev_in = [jax.device_put(a, sh) for a in concat_in]

        @jax.jit
        def _mkzeros():
            return tuple(
                jnp.zeros((n_cores * s[0], *s[1:]), d) for s, d in zero_shapes
            )

        _mkzeros = jax.jit(_mkzeros, out_shardings=(sh,) * n_outs)
        times = []
        for i in range(warmup + n):
            z = jax.block_until_ready(_mkzeros())
            t0 = _time.perf_counter()
            outs = sharded(*dev_in, *z)
            jax.block_until_ready(outs)
            dt = _time.perf_counter() - t0
            if i >= warmup:
                times.append(dt)
        return times

    return runner, concat, timed


def _in_maps(x, guidance, Wq, bq, Wk, bk, Wv, bv):
    x = np.ascontiguousarray(x, dtype=np.float32)
    guidance = np.ascontiguousarray(guidance, dtype=np.float32)
    wqt = np.ascontiguousarray(np.asarray(Wq, dtype=np.float32).T)
    wkt = np.ascontiguousarray(np.asarray(Wk, dtype=np.float32).T)
    wvt = np.ascontiguousarray(np.asarray(Wv, dtype=np.float32).T)
    bq = np.ascontiguousarray(bq, dtype=np.float32)
    bk = np.ascontiguousarray(bk, dtype=np.float32)
    bv = np.ascontiguousarray(bv, dtype=np.float32)
    maps = []
    for core in range(8):
        b, half = core // 2, core % 2
        xb_c = (
            x[b, half * (S // 2) : (half + 1) * (S // 2)] if SPLIT_S else x[b]
        )
        maps.append(
            {
                "xb": np.ascontiguousarray(xb_c),
                "gb": np.ascontiguousarray(guidance[b, half * LC : (half + 1) * LC]),
                "wqt": wqt,
                "wkt": wkt,
                "wvt": wvt,
                "bq": bq,
                "bk": bk,
                "bv": bv,
            }
        )
    return maps


def _gather(results):
    B = 4
    out = np.empty((B, 2 * LC, C), dtype=np.float32)
    for core in range(8):
        b, half = core // 2, core % 2
        out[b, half * LC : (half + 1) * LC] = results[core]["outb"]
    return out


def run(inputs, reps=1):
    with_bias = bool(
        np.any(inputs["bq"]) or np.any(inputs["bk"]) or np.any(inputs["bv"])
    )
    nc = _get_nc(reps, with_bias)
    key = ("runner", reps, with_bias, SPLIT_S, SKIP_CC)
    if key not in _CACHE:
        _CACHE[key] = _make_runner(nc)
    runner, concat, timed = _CACHE[key]
    maps = _in_maps(**inputs)
    return runner, timed, concat(maps)


def kernel(**inputs):
    runner, _, concat_in = run(inputs)
    return _gather(runner(concat_in))



# revision 27
# speedup vs baseline: 7.3149x; 7.3149x over previous
# Trainium2 Bass kernel for nn_CrossAttentionLayer (linear attention with
# elu+1 feature map).
#
# Math (per batch n):
#   q = guidance @ Wq.T + bq ; k = x @ Wk.T + bk ; v = x @ Wv.T + bv
#   Q = elu(q)+1 ; K = elu(k)+1          (per head h, head dim D=64)
#   KV_h = K_h^T @ (v_h/S);  Z = 1/(Q_h . sum_s K_h + eps)
#   out_h = (Q_h @ KV_h) * Z * S         (the /S and *S cancel exactly)
#
# Sharding: 8 cores = batch(4) x halves(2). Core pair (b, 0/1) splits the
# source sequence S=4096 in half for K/V/KV/Ksum (AllReduce of partial
# KV+Ksum within the pair) and splits the guidance rows in half for Q/out.
#
# v2 design notes (vs the fp32r baseline):
#  * All matmul operands are bf16 (host converts x/guidance/weights). bf16
#    runs 1 cycle/row at ANY moving width (fp32r needs >=256 cols) and
#    transposes at 1.0 cycles/row vs 2.0 for fp32. PSUM accumulation stays
#    fp32. Measured end-to-end rel err ~3e-3 (budget 2e-2).
#  * KV is built per head-PAIR: 4 matmuls of [128 tok, 128 kd] x
#    [128 tok, 130] where cols 0:128 are the pair's v-dims, col 128 is a
#    ones column that accumulates Ksum feature-major for free, col 129 pads
#    even. Off-diagonal 64x64 blocks are zeroed after the AllReduce.
#  * Output per l-tile: pd (denominators) via a blockdiag Ksum matmul
#    [P,8], and po via 4 pair matmuls of 128 cols each into ONE psum bank
#    (vs a 512-wide blockdiag stream) -> 4x less PE time on this stage.
#  * The AllReduce payload is bf16 [128, 520] (133KB) and is launched
#    right after phase 1; ALL of phase 2's g-transposes + q-projection +
#    elu (~20us of PE work) are emitted before anything reads the CC
#    result, so the collective is hidden off the critical path.
#  * Engine balance: exp/relu on ACT, q-side min on Pool (GPSIMD), the
#    rest of the elementwise work on DVE (bf16 SBUF ops run at 2-4x).

import sys

import numpy as np

if "/opt/trn_rl_repo" not in sys.path:
    sys.path.insert(0, "/opt/trn_rl_repo")

import ml_dtypes

import concourse.bacc as bacc
import concourse.mybir as mybir
import concourse.tile as tile
from concourse.masks import make_identity

P = 128
S = 4096
LC = 2048  # guidance rows per core
C = 512
H = 8
D = 64
NCT = C // P  # 4 column tiles
NST = S // P  # 32 s-tiles
EPS = 1e-6
SPLIT_S = True  # s-split across core pairs + AllReduce of partial KV/Ksum
SKIP_CC = False  # timing experiment: skip the collective (WRONG results)

F32 = mybir.dt.float32
BF = mybir.dt.bfloat16
NBF = ml_dtypes.bfloat16

Exp = mybir.ActivationFunctionType.Exp
Relu = mybir.ActivationFunctionType.Relu
Copy = mybir.ActivationFunctionType.Copy
Max = mybir.AluOpType.max
Mult = mybir.AluOpType.mult


def _build_nc(reps=1, with_bias=False, split_s=None):
    if split_s is None:
        split_s = SPLIT_S
    nc = bacc.Bacc(
        "TRN2",
        target_bir_lowering=False,
        debug=False,
        enable_asserts=False,
        num_devices=8,
    )
    xs = S // 2 if split_s else S
    xb = nc.dram_tensor("xb", [xs, C], BF, kind="ExternalInput").ap()
    gb = nc.dram_tensor("gb", [LC, C], BF, kind="ExternalInput").ap()
    wkt = nc.dram_tensor("wkt", [C, C], BF, kind="ExternalInput").ap()
    wvt = nc.dram_tensor("wvt", [C, C], BF, kind="ExternalInput").ap()
    wqt = nc.dram_tensor("wqt", [C, C], BF, kind="ExternalInput").ap()
    bk = nc.dram_tensor("bk", [C], BF, kind="ExternalInput").ap()
    bv = nc.dram_tensor("bv", [C], BF, kind="ExternalInput").ap()
    bq = nc.dram_tensor("bq", [C], F32, kind="ExternalInput").ap()
    outb = nc.dram_tensor("outb", [LC, C], F32, kind="ExternalOutput").ap()

    with tile.TileContext(nc) as tc:
        for rep in range(reps):
            _emit(nc, tc, xb, gb, wkt, wvt, wqt, bk, bv, bq, outb, rep=rep,
                  with_bias=with_bias, split_s=split_s)

    nc.compile()
    return nc


def _emit(nc, tc, xb, gb, wkt, wvt, wqt, bk, bv, bq, outb, rep=0,
          with_bias=False, split_s=False):
    mm = nc.tensor.matmul
    nst = NST // 2 if split_s else NST
    with tc.tile_pool(name=f"persist{rep}", bufs=1) as pp:
        # --- prefetch ALL x and g tiles into persistent SBUF, x first ---
        # SP/HWDGE issues these back to back; compute tiles never wait on
        # pool-buffer recycling and the SP queue never head-blocks.
        xts, gts = [], []
        for sg in range(nst // 2):
            xt = pp.tile([P, 2, C], BF, name=f"xt{rep}_{sg}")
            nc.sync.dma_start(
                xt, xb.rearrange("(g j p) c -> p g j c", j=2, p=P)[:, sg]
            )
            xts.append(xt)
        for gg in range(8):
            gt = pp.tile([P, 2, C], BF, name=f"gt{rep}_{gg}")
            nc.sync.dma_start(
                gt, gb.rearrange("(g j p) c -> p g j c", j=2, p=P)[:, gg]
            )
            gts.append(gt)

        # --- constants / weights resident in SBUF (all bf16 direct DMA) ---
        ident = pp.tile([P, P], BF)
        make_identity(nc, ident)
        # wk on the (still idle) ACT/HWDGE queue so the first projection can
        # start early; wv/wq on Pool's software-DGE path (separate from the
        # serial HWDGE descriptor fetcher)
        wk_sb = pp.tile([P, NCT, C], BF)
        wv_sb = pp.tile([P, NCT, C], BF)
        wq_sb = pp.tile([P, NCT, C], BF)
        nc.scalar.dma_start(wk_sb, wkt.rearrange("(t p) n -> p t n", p=P))
        nc.gpsimd.dma_start(wv_sb, wvt.rearrange("(t p) n -> p t n", p=P))
        nc.gpsimd.dma_start(wq_sb, wqt.rearrange("(t p) n -> p t n", p=P))
        if with_bias:
            ones_row = pp.tile([1, P], BF)
            nc.vector.memset(ones_row, 1.0)
            bk_row = pp.tile([1, C], BF)
            bv_row = pp.tile([1, C], BF)
            nc.gpsimd.dma_start(bk_row, bk.rearrange("(a c) -> a c", a=1))
            nc.gpsimd.dma_start(bv_row, bv.rearrange("(a c) -> a c", a=1))
        bqT = pp.tile([P, NCT], F32)
        nc.gpsimd.dma_start(bqT, bq.rearrange("(t p) -> p t", p=P))
        bqT1 = pp.tile([P, NCT], F32)
        nc.vector.tensor_scalar_add(bqT1, bqT, 1.0)
        onezero = pp.tile([P, 2], BF)
        nc.vector.memset(onezero[:, 0:1], 1.0)
        nc.vector.memset(onezero[:, 1:2], 0.0)

        # blockdiag KV pair blocks [cin_part, pair, 128] and blockdiag Ksum
        kvbd = pp.tile([P, NCT, P], BF)
        ksbd = pp.tile([P, NCT, H], BF)
        nc.vector.memset(kvbd, 0.0)
        nc.vector.memset(ksbd, 0.0)
        stgb = pp.tile([P, 4, 130], BF)
        stg2 = pp.tile([P, 4, 130], BF)

        # ---------------- phase 1: x -> K,V -> KV, Ksum ----------------
        # Two 128-token tiles per pipeline stage: both transposes land in one
        # psum bank and the psum->sbuf copies / elu / v-staging run as single
        # wide (1024-col) ops, halving per-instruction fixed costs.
        # Per head-pair p: KV accumulated in psum [128 kd, 130] where cols
        # 0:128 are the pair's v dims, col 128 accumulates Ksum (ones col
        # in the moving operand), col 129 pads the free dim even. Two pairs
        # share each psum bank (first matmul's start=True zeroes the bank).
        with (
            tc.tile_pool(name=f"p1_{rep}", bufs=3) as p1,
            tc.tile_pool(name=f"p1ps_{rep}", bufs=4, space="PSUM") as p1ps,
            tc.tile_pool(name=f"tps_{rep}", bufs=2, space="PSUM") as tps,
            tc.tile_pool(name=f"accps_{rep}", bufs=1, space="PSUM") as accps,
        ):
            kv_ps = [
                accps.tile([P, 2, 130], F32, tag=f"kv{b}", name=f"kv_ps{b}")
                for b in range(2)
            ]

            def feature_map(stage):
                # K = elu(k)+1 = max(relu(k+1), min(exp(k), 1)); also stage
                # V (+ ones column for Ksum) into bf16 SBUF
                pks, pvs = stage
                e_sb = p1.tile([P, 2, C], BF, tag="e")
                u_sb = p1.tile([P, 2, C], BF, tag="u")
                for j in (0, 1):
                    nc.scalar.activation(e_sb[:, j, :], pks[j], Exp)
                    nc.scalar.activation(u_sb[:, j, :], pks[j], Relu, bias=1.0)
                nc.vector.tensor_scalar_min(e_sb, e_sb, 1.0)
                k_sb = p1.tile([P, 2, C], BF, tag="k")
                nc.vector.tensor_tensor(k_sb, e_sb, u_sb, Max)
                v_ext = p1.tile([P, 2, NCT, 130], BF, tag="v")
                for j in (0, 1):
                    nc.vector.tensor_copy(
                        v_ext[:, j, :, 0:P],
                        pvs[j].rearrange("p (a c) -> p a c", c=P),
                    )
                nc.vector.tensor_copy(
                    v_ext[:, :, :, P : P + 2],
                    onezero[:, None, None, :].to_broadcast([P, 2, NCT, 2]),
                )
                return k_sb, v_ext

            def kv_update(kv, first, last):
                k_sb, v_ext = kv
                for j in (0, 1):
                    for hp in range(4):
                        mm(kv_ps[hp // 2][:, hp % 2, :],
                           k_sb[:, j, hp * P : (hp + 1) * P],
                           v_ext[:, j, hp, :],
                           start=(first and j == 0 and hp % 2 == 0),
                           stop=(last and j == 1 and hp % 2 == 1))

            prev_stage = None
            prev_kv = None
            for sg in range(nst // 2):
                xt = xts[sg]
                pt = tps.tile([P, 2, NCT, P], BF, tag="tp")
                for j in (0, 1):
                    for ci in range(NCT):
                        mm(pt[:, j, ci, :], xt[:, j, ci * P : (ci + 1) * P],
                           ident, is_transpose=True,
                           start=(j == 0 and ci == 0),
                           stop=(j == 1 and ci == NCT - 1))
                xT = p1.tile([P, 2, NCT, P], BF, tag="xT")
                nc.vector.tensor_copy(xT, pt)
                pks, pvs = [], []
                for j in (0, 1):
                    # k projection (token-major): psum[s,cout]
                    pk = p1ps.tile([P, C], F32, tag="proj")
                    if with_bias:
                        mm(pk, ones_row, bk_row, start=True, stop=False)
                    for ci in range(NCT):
                        mm(pk, xT[:, j, ci, :], wk_sb[:, ci, :],
                           start=(ci == 0 and not with_bias),
                           stop=(ci == NCT - 1))
                    # v projection
                    pv = p1ps.tile([P, C], F32, tag="proj")
                    if with_bias:
                        mm(pv, ones_row, bv_row, start=True, stop=False)
                    for ci in range(NCT):
                        mm(pv, xT[:, j, ci, :], wv_sb[:, ci, :],
                           start=(ci == 0 and not with_bias),
                           stop=(ci == NCT - 1))
                    pks.append(pk)
                    pvs.append(pv)
                # two-level software pipeline: feature-map the PREVIOUS
                # stage's psum; the KV matmuls run a further stage behind so
                # they never wait on the DVE chain
                if prev_kv is not None:
                    kv_update(prev_kv, sg == 2, False)
                if prev_stage is not None:
                    prev_kv = feature_map(prev_stage)
                prev_stage = (pks, pvs)
            kv_update(prev_kv, nst == 4, False)
            prev_kv = feature_map(prev_stage)
            kv_update(prev_kv, False, True)

            # pack partial KV(+Ksum) for the pair AllGather (bf16 wire)
            nc.vector.tensor_copy(stgb[:, 0:2, :], kv_ps[0])
            nc.vector.tensor_copy(stgb[:, 2:4, :], kv_ps[1])

        # AllGather the pair's partial KV/Ksum (bf16, 133KB) and add the two
        # halves locally on DVE: cheaper than AllReduce (one-way transfer).
        ccin = nc.dram_tensor(f"ccin{rep}", [P, 520], BF).ap()
        ccout = nc.dram_tensor(f"ccout{rep}", [2, P, 520], BF).ap()
        nc.gpsimd.dma_start(ccin.rearrange("p (a v) -> p a v", a=4), stgb)
        do_cc = split_s and not SKIP_CC
        if do_cc:
            nc.gpsimd.collective_compute(
                "AllGather",
                mybir.AluOpType.bypass,
                replica_groups=[[0, 1], [2, 3], [4, 5], [6, 7]],
                ins=[ccin],
                outs=[ccout],
            )

        # ---------------- phase 2: guidance -> Q -> out ----------------
        # All 4 chunks of g-transpose + q-projection + elu are emitted
        # BEFORE the collective result is read, hiding the AllReduce.
        qTs = [pp.tile([P, NCT, C], BF, name=f"qT{rep}_{lc}") for lc in range(4)]
        with (
            tc.tile_pool(name=f"p2_{rep}", bufs=3) as p2,
            tc.tile_pool(name=f"p2ps_{rep}", bufs=2, space="PSUM") as p2ps,
            tc.tile_pool(name=f"tps2_{rep}", bufs=2, space="PSUM") as tps2,
            tc.tile_pool(name=f"pops_{rep}", bufs=2, space="PSUM") as pops,
            tc.tile_pool(name=f"dps_{rep}", bufs=2, space="PSUM") as dps,
        ):
            for lc in range(4):
                gT = p2.tile([P, NCT, C], BF, tag="gT")
                for half in (0, 1):
                    gt = gts[lc * 2 + half]
                    pt2 = tps2.tile([P, 2, NCT, P], BF, tag="tp2")
                    for j in (0, 1):
                        for ci in range(NCT):
                            mm(pt2[:, j, ci, :],
                               gt[:, j, ci * P : (ci + 1) * P], ident,
                               is_transpose=True,
                               start=(j == 0 and ci == 0),
                               stop=(j == 1 and ci == NCT - 1))
                    nc.vector.tensor_copy(
                        gT[:, :, half * 2 * P : (half * 2 + 2) * P].rearrange(
                            "p a (j q) -> p j a q", j=2
                        ),
                        pt2,
                    )
                # q projection, feature-major: psum[cout, l]
                for ct in range(NCT):
                    pq = p2ps.tile([P, C], F32, tag="pq")
                    for ci in range(NCT):
                        mm(pq, wq_sb[:, ci, ct * P : (ct + 1) * P],
                           gT[:, ci, :], start=(ci == 0), stop=(ci == NCT - 1))
                    e2 = p2.tile([P, C], BF, tag="e2")
                    nc.scalar.activation(e2, pq, Exp, bias=bqT[:, ct : ct + 1])
                    u2 = p2.tile([P, C], BF, tag="u2")
                    nc.scalar.activation(u2, pq, Relu, bias=bqT1[:, ct : ct + 1])
                    nc.vector.tensor_scalar_min(e2, e2, 1.0)
                    nc.vector.tensor_tensor(qTs[lc][:, ct, :], e2, u2, Max)

            # unpack the gathered partial KV/Ksum: add the two ranks' halves,
            # then scatter into blockdiag operands (pair block p: diag 64x64
            # head blocks kept, off-diag stays 0; Ksum col 128 into ksbd).
            if do_cc:
                stg2ab = pp.tile([P, 2, NCT, 130], BF)
                nc.sync.dma_start(
                    stg2ab, ccout.rearrange("r p (a v) -> p r a v", a=NCT)
                )
                nc.vector.tensor_tensor(
                    stg2, stg2ab[:, 0], stg2ab[:, 1], mybir.AluOpType.add
                )
            else:
                nc.sync.dma_start(
                    stg2, ccin.rearrange("p (a v) -> p a v", a=NCT)
                )
            nc.vector.tensor_copy(kvbd[0:D, :, 0:D], stg2[0:D, :, 0:D])
            nc.vector.tensor_copy(kvbd[D:P, :, D:P], stg2[D:P, :, D:P])
            ksf = ksbd.rearrange("p a h -> p (a h)")
            nc.vector.tensor_copy(
                ksf[0:D, 0 : NCT * H : H + 2], stg2[0:D, :, P]
            )
            nc.vector.tensor_copy(
                ksf[D:P, 1 : NCT * H : H + 2], stg2[D:P, :, P]
            )

            for lc in range(4):
                # denominators for the whole 512-row chunk in one psum bank
                # (one eps-add + one reciprocal instead of four)
                pd = dps.tile([P, 4, H], F32, tag="pd")
                for lt in range(4):
                    lsl = slice(lt * P, (lt + 1) * P)
                    for ct in range(NCT):
                        mm(pd[:, lt, :], qTs[lc][:, ct, lsl], ksbd[:, ct, :],
                           start=(lt == 0 and ct == 0),
                           stop=(lt == 3 and ct == NCT - 1))
                zl = p2.tile([P, 4, H], F32, tag="zl")
                nc.scalar.activation(zl, pd, Copy, bias=EPS)
                nc.vector.reciprocal(zl, zl)
                for lt in range(4):
                    lsl = slice(lt * P, (lt + 1) * P)
                    po = pops.tile([P, NCT, P], F32, tag="po")
                    for hp in range(NCT):
                        mm(po[:, hp, :], qTs[lc][:, hp, lsl], kvbd[:, hp, :],
                           start=(hp == 0), stop=(hp == NCT - 1))
                    # out = po * z broadcast; heads 0-5 on DVE, 6-7 on ACT
                    osb = p2.tile([P, C], F32, tag="osb")
                    nc.vector.tensor_tensor(
                        osb[:, 0 : 6 * D].rearrange("p (h v) -> p h v", h=6),
                        po.rearrange("p a (b v) -> p (a b) v", b=2)[:, 0:6],
                        zl[:, lt, 0:6, None].to_broadcast([P, 6, D]),
                        Mult,
                    )
                    for h in (6, 7):
                        nc.scalar.activation(
                            osb[:, h * D : (h + 1) * D],
                            po[:, 3, (h - 6) * D : (h - 5) * D],
                            Copy,
                            scale=zl[:, lt, h : h + 1],
                        )
                    # alternate DMA queues so the final drain isn't serialized
                    # behind one queue's per-DMA issue overhead
                    eng = nc.sync if lt % 2 == 0 else nc.scalar
                    eng.dma_start(
                        outb[(lc * 4 + lt) * P : (lc * 4 + lt + 1) * P, :], osb
                    )


_CACHE = {}


def _get_nc(reps=1, with_bias=False):
    key = ("nc", reps, with_bias, SPLIT_S, SKIP_CC)
    if key not in _CACHE:
        _CACHE[key] = _build_nc(reps, with_bias, SPLIT_S)
    return _CACHE[key]


def _make_runner(nc):
    """Build a reusable jitted SPMD runner for `nc` (mirrors
    bass2jax.run_bass_via_pjrt's multi-core branch, but caches the jit so
    repeated calls don't re-lower/re-compile)."""
    import jax
    from jax.sharding import Mesh, PartitionSpec
    from jax.experimental.shard_map import shard_map

    import concourse.mybir as mb
    from concourse import bass2jax

    bass2jax.install_neuronx_cc_hook()

    n_cores = 8
    partition_name = (
        nc.partition_id_tensor.name if nc.partition_id_tensor else None
    )
    in_names, out_names, out_avals, zero_shapes = [], [], [], []
    for alloc in nc.m.functions[0].allocations:
        if not isinstance(alloc, mb.MemoryLocationSet):
            continue
        name = alloc.memorylocations[0].name
        if alloc.kind == "ExternalInput":
            if name != partition_name:
                in_names.append(name)
        elif alloc.kind == "ExternalOutput":
            shape = tuple(alloc.tensor_shape)
            dtype = mb.dt.np(alloc.dtype)
            out_names.append(name)
            out_avals.append(jax.core.ShapedArray(shape, dtype))
            zero_shapes.append((shape, dtype))
    n_params = len(in_names)
    n_outs = len(out_names)
    all_names = in_names + out_names
    if partition_name is not None:
        all_names.append(partition_name)
    donate = tuple(range(n_params, n_params + n_outs))

    def _body(*args):
        operands = list(args)
        if partition_name is not None:
            operands.append(bass2jax.partition_id_tensor())
        outs = bass2jax._bass_exec_p.bind(
            *operands,
            out_avals=tuple(out_avals),
            in_names=tuple(all_names),
            out_names=tuple(out_names),
            lowering_input_output_aliases=(),
            sim_require_finite=True,
            sim_require_nnan=True,
            nc=nc,
        )
        return tuple(outs)

    devices = jax.devices()[:n_cores]
    mesh = Mesh(np.asarray(devices), ("core",))
    in_specs = (PartitionSpec("core"),) * (n_params + n_outs)
    out_specs = (PartitionSpec("core"),) * n_outs
    sharded = jax.jit(
        shard_map(
            _body, mesh=mesh, in_specs=in_specs, out_specs=out_specs,
            check_rep=False,
        ),
        donate_argnums=donate,
        keep_unused=True,
    )

    def _zeros():
        return [
            np.zeros((n_cores * sh[0], *sh[1:]), dt) for sh, dt in zero_shapes
        ]

    def runner(concat_in):
        out_arrs = sharded(*concat_in, *_zeros())
        return [
            {
                name: np.asarray(out_arrs[i]).reshape(
                    n_cores, *out_avals[i].shape
                )[c]
                for i, name in enumerate(out_names)
            }
            for c in range(n_cores)
        ]

    def concat(maps):
        return [
            np.concatenate([np.asarray(m[name]) for m in maps], axis=0)
            for name in in_names
        ]

    def timed(concat_in, n=10, warmup=2):
        """Time `n` executions with device-resident inputs and on-device
        donated zero outputs, so per-call host traffic is ~zero."""
        import time as _time
        import jax.numpy as jnp
        from jax.sharding import NamedSharding

        sh = NamedSharding(mesh, PartitionSpec("core"))
        dev_in = [jax.device_put(a, sh) for a in concat_in]

        @jax.jit
        def _mkzeros():
            return tuple(
                jnp.zeros((n_cores * s[0], *s[1:]), d) for s, d in zero_shapes
            )

        _mkzeros = jax.jit(_mkzeros, out_shardings=(sh,) * n_outs)
        times = []
        for i in range(warmup + n):
            z = jax.block_until_ready(_mkzeros())
            t0 = _time.perf_counter()
            outs = sharded(*dev_in, *z)
            jax.block_until_ready(outs)
            dt = _time.perf_counter() - t0
            if i >= warmup:
                times.append(dt)
        return times

    return runner, concat, timed


def _in_maps(x, guidance, Wq, bq, Wk, bk, Wv, bv):
    x = np.asarray(x, dtype=np.float32).astype(NBF)
    guidance = np.asarray(guidance, dtype=np.float32).astype(NBF)
    wqt = np.ascontiguousarray(np.asarray(Wq, dtype=np.float32).T).astype(NBF)
    wkt = np.ascontiguousarray(np.asarray(Wk, dtype=np.float32).T).astype(NBF)
    wvt = np.ascontiguousarray(np.asarray(Wv, dtype=np.float32).T).astype(NBF)
    bq = np.ascontiguousarray(bq, dtype=np.float32)
    bk = np.asarray(bk, dtype=np.float32).astype(NBF)
    bv = np.asarray(bv, dtype=np.float32).astype(NBF)
    maps = []
    for core in range(8):
        b, half = core // 2, core % 2
        xb_c = (
            x[b, half * (S // 2) : (half + 1) * (S // 2)] if SPLIT_S else x[b]
        )
        maps.append(
            {
                "xb": np.ascontiguousarray(xb_c),
                "gb": np.ascontiguousarray(
                    guidance[b, half * LC : (half + 1) * LC]
                ),
                "wqt": wqt,
                "wkt": wkt,
                "wvt": wvt,
                "bq": bq,
                "bk": bk,
                "bv": bv,
            }
        )
    return maps


def _gather(results):
    B = 4
    out = np.empty((B, 2 * LC, C), dtype=np.float32)
    for core in range(8):
        b, half = core // 2, core % 2
        out[b, half * LC : (half + 1) * LC] = results[core]["outb"]
    return out


def run(inputs, reps=1):
    with_bias = bool(
        np.any(inputs["bq"]) or np.any(inputs["bk"]) or np.any(inputs["bv"])
    )
    nc = _get_nc(reps, with_bias)
    key = ("runner", reps, with_bias, SPLIT_S, SKIP_CC)
    if key not in _CACHE:
        _CACHE[key] = _make_runner(nc)
    runner, concat, timed = _CACHE[key]
    maps = _in_maps(**inputs)
    return runner, timed, concat(maps)


def kernel(**inputs):
    runner, _, concat_in = run(inputs)
    return _gather(runner(concat_in))


# revision 40
# speedup vs baseline: 320.4261x; 43.8048x over previous
# Trainium2 Bass kernel for nn_CrossAttentionLayer (linear attention with
# elu+1 feature map).
#
# Math (per batch n):
#   q = guidance @ Wq.T + bq ; k = x @ Wk.T + bk ; v = x @ Wv.T + bv
#   Q = elu(q)+1 ; K = elu(k)+1          (per head h, head dim D=64)
#   KV_h = K_h^T @ (v_h/S);  Z = 1/(Q_h . sum_s K_h + eps)
#   out_h = (Q_h @ KV_h) * Z * S         (the /S and *S cancel exactly)
#
# Sharding: 8 cores = batch(4) x halves(2). Core pair (b, 0/1) splits the
# source sequence S=4096 in half for K/V/KV/Ksum (AllReduce of partial
# KV+Ksum within the pair) and splits the guidance rows in half for Q/out.
#
# v2 design notes (vs the fp32r baseline):
#  * All matmul operands are bf16 (host converts x/guidance/weights). bf16
#    runs 1 cycle/row at ANY moving width (fp32r needs >=256 cols) and
#    transposes at 1.0 cycles/row vs 2.0 for fp32. PSUM accumulation stays
#    fp32. Measured end-to-end rel err ~3e-3 (budget 2e-2).
#  * KV is built per head-PAIR: 4 matmuls of [128 tok, 128 kd] x
#    [128 tok, 130] where cols 0:128 are the pair's v-dims, col 128 is a
#    ones column that accumulates Ksum feature-major for free, col 129 pads
#    even. Off-diagonal 64x64 blocks are zeroed after the AllReduce.
#  * Output per l-tile: pd (denominators) via a blockdiag Ksum matmul
#    [P,8], and po via 4 pair matmuls of 128 cols each into ONE psum bank
#    (vs a 512-wide blockdiag stream) -> 4x less PE time on this stage.
#  * The AllReduce payload is bf16 [128, 520] (133KB) and is launched
#    right after phase 1; ALL of phase 2's g-transposes + q-projection +
#    elu (~20us of PE work) are emitted before anything reads the CC
#    result, so the collective is hidden off the critical path.
#  * Engine balance: exp/relu on ACT, q-side min on Pool (GPSIMD), the
#    rest of the elementwise work on DVE (bf16 SBUF ops run at 2-4x).

import sys

import numpy as np

if "/opt/trn_rl_repo" not in sys.path:
    sys.path.insert(0, "/opt/trn_rl_repo")

import ml_dtypes

import concourse.bacc as bacc
import concourse.mybir as mybir
import concourse.tile as tile
from concourse.masks import make_identity

P = 128
S = 4096
LC = 2048  # guidance rows per core
C = 512
H = 8
D = 64
NCT = C // P  # 4 column tiles
NST = S // P  # 32 s-tiles
EPS = 1e-6
SPLIT_S = True  # s-split across core pairs + AllReduce of partial KV/Ksum
SKIP_CC = False  # timing experiment: skip the collective (WRONG results)

F32 = mybir.dt.float32
BF = mybir.dt.bfloat16
NBF = ml_dtypes.bfloat16

Exp = mybir.ActivationFunctionType.Exp
Relu = mybir.ActivationFunctionType.Relu
Copy = mybir.ActivationFunctionType.Copy
Max = mybir.AluOpType.max
Mult = mybir.AluOpType.mult


def _build_nc(reps=1, with_bias=False, split_s=None):
    if split_s is None:
        split_s = SPLIT_S
    nc = bacc.Bacc(
        "TRN2",
        target_bir_lowering=False,
        debug=False,
        enable_asserts=False,
        num_devices=8,
    )
    xs = S // 2 if split_s else S
    xb = nc.dram_tensor("xb", [xs, C], BF, kind="ExternalInput").ap()
    gb = nc.dram_tensor("gb", [LC, C], BF, kind="ExternalInput").ap()
    wkt = nc.dram_tensor("wkt", [C, C], BF, kind="ExternalInput").ap()
    wvt = nc.dram_tensor("wvt", [C, C], BF, kind="ExternalInput").ap()
    wqt = nc.dram_tensor("wqt", [C, C], BF, kind="ExternalInput").ap()
    bk = nc.dram_tensor("bk", [C], BF, kind="ExternalInput").ap()
    bv = nc.dram_tensor("bv", [C], BF, kind="ExternalInput").ap()
    bq = nc.dram_tensor("bq", [C], F32, kind="ExternalInput").ap()
    outb = nc.dram_tensor("outb", [LC, C], BF, kind="ExternalOutput").ap()

    with tile.TileContext(nc) as tc:
        for rep in range(reps):
            _emit(nc, tc, xb, gb, wkt, wvt, wqt, bk, bv, bq, outb, rep=rep,
                  with_bias=with_bias, split_s=split_s)

    nc.compile()
    return nc


def _emit(nc, tc, xb, gb, wkt, wvt, wqt, bk, bv, bq, outb, rep=0,
          with_bias=False, split_s=False):
    mm = nc.tensor.matmul
    nst = NST // 2 if split_s else NST
    with tc.tile_pool(name=f"persist{rep}", bufs=1) as pp:
        # --- prefetch ALL x and g tiles into persistent SBUF, x first ---
        # SP/HWDGE issues these back to back; compute tiles never wait on
        # pool-buffer recycling and the SP queue never head-blocks.
        xts, gts = [], []
        for sg in range(nst // 2):
            xt = pp.tile([P, 2, C], BF, name=f"xt{rep}_{sg}")
            src_ap = xb.rearrange("(g j p) c -> p g j c", j=2, p=P)[:, sg]
            if sg == 0:
                nc.sync.dma_start(xt[:, 0], src_ap[:, 0])
                nc.sync.dma_start(xt[:, 1], src_ap[:, 1])
            else:
                nc.sync.dma_start(xt, src_ap)
            xts.append(xt)
        for gg in range(8):
            gt = pp.tile([P, 2, C], BF, name=f"gt{rep}_{gg}")
            nc.sync.dma_start(
                gt, gb.rearrange("(g j p) c -> p g j c", j=2, p=P)[:, gg]
            )
            gts.append(gt)

        # --- constants / weights resident in SBUF (all bf16 direct DMA) ---
        ident = pp.tile([P, P], BF)
        make_identity(nc, ident)
        # wk on the (still idle) ACT/HWDGE queue so the first projection can
        # start early; wv/wq on Pool's software-DGE path (separate from the
        # serial HWDGE descriptor fetcher)
        wk_sb = pp.tile([P, NCT, C], BF)
        wv_sb = pp.tile([P, NCT, C], BF)
        wq_sb = pp.tile([P, NCT, C], BF)
        nc.scalar.dma_start(wk_sb, wkt.rearrange("(t p) n -> p t n", p=P))
        nc.gpsimd.dma_start(wv_sb, wvt.rearrange("(t p) n -> p t n", p=P))
        nc.gpsimd.dma_start(wq_sb, wqt.rearrange("(t p) n -> p t n", p=P))
        if with_bias:
            ones_row = pp.tile([1, P], BF)
            nc.vector.memset(ones_row, 1.0)
            bk_row = pp.tile([1, C], BF)
            bv_row = pp.tile([1, C], BF)
            nc.gpsimd.dma_start(bk_row, bk.rearrange("(a c) -> a c", a=1))
            nc.gpsimd.dma_start(bv_row, bv.rearrange("(a c) -> a c", a=1))
        bqT = pp.tile([P, NCT], F32)
        nc.gpsimd.dma_start(bqT, bq.rearrange("(t p) -> p t", p=P))
        bqT1 = pp.tile([P, NCT], F32)
        nc.vector.tensor_scalar_add(bqT1, bqT, 1.0)
        # ones columns for the Ksum matmuls: row half r=0 accumulates into
        # psum col 128, r=1 into col 129 (so the two heads' Ksums stay
        # separate in the pair block)
        onezero = pp.tile([P, 2], BF)
        nc.vector.memset(onezero[:, 0:1], 1.0)
        nc.vector.memset(onezero[:, 1:2], 0.0)
        stgb = pp.tile([P, 4, 130], BF)
        # blockdiag KV pair blocks [cin_part, pair, 128] and blockdiag Ksum
        kvbd = pp.tile([P, NCT, P], BF)
        ksbd = pp.tile([P, NCT, H], BF)
        nc.vector.memset(kvbd, 0.0)
        nc.vector.memset(ksbd, 0.0)
        stg2 = pp.tile([P, 4, 130], BF)

        # PE p-state warm-up: ~45 dependency-free 128-col transposes of a
        # zeroed tile keep the PE continuously busy from t~0 so the real
        # matmuls start at full clock right as the first x tile lands.
        warm = pp.tile([P, P], BF)
        nc.vector.memset(warm, 0.0)
        with tc.tile_pool(name=f"warm{rep}", bufs=1, space="PSUM") as wps:
            wp = wps.tile([P, P], BF)
            for wi in range(45):
                mm(wp, warm, warm, is_transpose=True,
                   start=(wi == 0), stop=(wi == 44))

        # ---------------- phase 1: x -> K,V -> KV, Ksum ----------------
        # Two 128-token tiles per pipeline stage: both transposes land in one
        # psum bank and the psum->sbuf copies / elu / v-staging run as single
        # wide (1024-col) ops, halving per-instruction fixed costs.
        # Per head-pair p: KV accumulated in psum [128 kd, 130] where cols
        # 0:128 are the pair's v dims, col 128 accumulates Ksum (ones col
        # in the moving operand), col 129 pads the free dim even. Two pairs
        # share each psum bank (first matmul's start=True zeroes the bank).
        with (
            tc.tile_pool(name=f"p1_{rep}", bufs=3) as p1,
            tc.tile_pool(name=f"p1ps_{rep}", bufs=4, space="PSUM") as p1ps,
            tc.tile_pool(name=f"tps_{rep}", bufs=2, space="PSUM") as tps,
            tc.tile_pool(name=f"accps_{rep}", bufs=1, space="PSUM") as accps,
        ):
            kv_ps = [
                accps.tile([P, 2, 130], F32, tag=f"kv{b}", name=f"kv_ps{b}")
                for b in range(2)
            ]

            def feature_map(stage):
                # K = elu(k)+1 = max(relu(k+1), min(exp(k), 1)); also stage
                # V (+ ones column for Ksum) into bf16 SBUF
                pks, pvs = stage
                e_sb = p1.tile([P, 2, C], BF, tag="e")
                u_sb = p1.tile([P, 2, C], BF, tag="u")
                for j in (0, 1):
                    nc.scalar.activation(e_sb[:, j, :], pks[j], Exp)
                    nc.scalar.activation(u_sb[:, j, :], pks[j], Relu, bias=1.0)
                nc.vector.tensor_scalar_min(e_sb, e_sb, 1.0)
                k_sb = p1.tile([P, 2, C], BF, tag="k")
                nc.vector.tensor_tensor(k_sb, e_sb, u_sb, Max)
                v_ext = p1.tile([P, 2, NCT, 130], BF, tag="v")
                nc.scalar.copy(
                    v_ext[:, 0, :, 0:P],
                    pvs[0].rearrange("p (a c) -> p a c", c=P),
                )
                nc.vector.tensor_copy(
                    v_ext[:, 1, :, 0:P],
                    pvs[1].rearrange("p (a c) -> p a c", c=P),
                )
                nc.vector.tensor_copy(
                    v_ext[:, :, :, P : P + 2],
                    onezero[:, None, None, :].to_broadcast([P, 2, NCT, 2]),
                )
                return k_sb, v_ext

            def kv_update(kv, first, last):
                k_sb, v_ext = kv
                for j in (0, 1):
                    for hp in range(4):
                        mm(kv_ps[hp // 2][:, hp % 2, :],
                           k_sb[:, j, hp * P : (hp + 1) * P],
                           v_ext[:, j, hp, :],
                           start=(first and j == 0 and hp % 2 == 0),
                           stop=(last and j == 1 and hp % 2 == 1))

            prev_stage = None
            prev_kv = None
            for sg in range(nst // 2):
                xt = xts[sg]
                pt = tps.tile([P, 2, NCT, P], BF, tag="tp")
                for j in (0, 1):
                    for ci in range(NCT):
                        mm(pt[:, j, ci, :], xt[:, j, ci * P : (ci + 1) * P],
                           ident, is_transpose=True,
                           start=(j == 0 and ci == 0),
                           stop=(j == 1 and ci == NCT - 1))
                xT = p1.tile([P, 2, NCT, P], BF, tag="xT")
                nc.vector.tensor_copy(xT, pt)
                pks, pvs = [], []
                for j in (0, 1):
                    # k projection (token-major): psum[s,cout]
                    pk = p1ps.tile([P, C], F32, tag="proj")
                    if with_bias:
                        mm(pk, ones_row, bk_row, start=True, stop=False)
                    for ci in range(NCT):
                        mm(pk, xT[:, j, ci, :], wk_sb[:, ci, :],
                           start=(ci == 0 and not with_bias),
                           stop=(ci == NCT - 1))
                    # v projection
                    pv = p1ps.tile([P, C], F32, tag="proj")
                    if with_bias:
                        mm(pv, ones_row, bv_row, start=True, stop=False)
                    for ci in range(NCT):
                        mm(pv, xT[:, j, ci, :], wv_sb[:, ci, :],
                           start=(ci == 0 and not with_bias),
                           stop=(ci == NCT - 1))
                    pks.append(pk)
                    pvs.append(pv)
                # two-level software pipeline: feature-map the PREVIOUS
                # stage's psum; the KV matmuls run a further stage behind so
                # they never wait on the DVE chain
                if prev_kv is not None:
                    kv_update(prev_kv, sg == 2, False)
                if prev_stage is not None:
                    prev_kv = feature_map(prev_stage)
                prev_stage = (pks, pvs)
            kv_update(prev_kv, nst == 4, False)
            prev_kv = feature_map(prev_stage)
            kv_update(prev_kv, False, True)

            # pack partial KV(+Ksum) for the pair AllGather (bf16 wire);
            # on ACT so the DVE queue flows straight into phase-2 prep
            nc.scalar.copy(stgb[:, 0:2, :], kv_ps[0])
            nc.scalar.copy(stgb[:, 2:4, :], kv_ps[1])

        # AllGather the pair's partial KV/Ksum (bf16, 133KB) and add the two
        # halves locally on DVE: cheaper than AllReduce (one-way transfer).
        ccin = nc.dram_tensor(f"ccin{rep}", [P, 520], BF).ap()
        ccout = nc.dram_tensor(f"ccout{rep}", [2, P, 520], BF).ap()
        nc.gpsimd.dma_start(ccin.rearrange("p (a v) -> p a v", a=4), stgb)
        do_cc = split_s and not SKIP_CC
        if do_cc:
            nc.gpsimd.collective_compute(
                "AllGather",
                mybir.AluOpType.bypass,
                replica_groups=[[0, 1], [2, 3], [4, 5], [6, 7]],
                ins=[ccin],
                outs=[ccout],
            )

        # ---------------- phase 2: guidance -> Q -> out ----------------
        # All 4 chunks of g-transpose + q-projection + elu are emitted
        # BEFORE the collective result is read, hiding the AllReduce.
        qTs = [pp.tile([P, NCT, C], BF, name=f"qT{rep}_{lc}") for lc in range(4)]
        with (
            tc.tile_pool(name=f"p2_{rep}", bufs=3) as p2,
            tc.tile_pool(name=f"osbp_{rep}", bufs=5) as osbp,
            tc.tile_pool(name=f"pops_{rep}", bufs=3, space="PSUM") as pops,
            tc.tile_pool(name=f"dps_{rep}", bufs=2, space="PSUM") as dps,
            tc.tile_pool(name=f"p2ps_{rep}", bufs=2, space="PSUM") as p2ps,
            tc.tile_pool(name=f"tps2_{rep}", bufs=1, space="PSUM") as tps2,
        ):
            for lc in range(4):
                gT = p2.tile([P, NCT, C], BF, tag="gT")
                for half in (0, 1):
                    gt = gts[lc * 2 + half]
                    pt2 = tps2.tile([P, 2, NCT, P], BF, tag="tp2")
                    for j in (0, 1):
                        for ci in range(NCT):
                            mm(pt2[:, j, ci, :],
                               gt[:, j, ci * P : (ci + 1) * P], ident,
                               is_transpose=True,
                               start=(j == 0 and ci == 0),
                               stop=(j == 1 and ci == NCT - 1))
                    nc.vector.tensor_copy(
                        gT[:, :, half * 2 * P : (half * 2 + 2) * P].rearrange(
                            "p a (j q) -> p j a q", j=2
                        ),
                        pt2,
                    )
                # q projection, feature-major: psum[cout, l]
                for ct in range(NCT):
                    pq = p2ps.tile([P, C], F32, tag="pq")
                    for ci in range(NCT):
                        mm(pq, wq_sb[:, ci, ct * P : (ct + 1) * P],
                           gT[:, ci, :], start=(ci == 0), stop=(ci == NCT - 1))
                    e2 = p2.tile([P, C], BF, tag="e2")
                    nc.scalar.activation(e2, pq, Exp, bias=bqT[:, ct : ct + 1])
                    u2 = p2.tile([P, C], BF, tag="u2")
                    nc.scalar.activation(u2, pq, Relu, bias=bqT1[:, ct : ct + 1])
                    nc.vector.tensor_scalar_min(e2, e2, 1.0)
                    nc.vector.tensor_tensor(qTs[lc][:, ct, :], e2, u2, Max)

            # unpack the gathered partial KV/Ksum: sum the two ranks'
            # halves DIRECTLY into the blockdiag operand views (pair block
            # p: diag 64x64 head blocks kept, off-diag stays 0; Ksum col
            # 128 scattered into ksbd) - 4 strided adds, no staging pass.
            ksf = ksbd.rearrange("p a h -> p (a h)")
            Add = mybir.AluOpType.add
            if do_cc:
                stg2ab = pp.tile([P, 2, NCT, 130], BF)
                nc.sync.dma_start(
                    stg2ab, ccout.rearrange("r p (a v) -> p r a v", a=NCT)
                )
                nc.vector.tensor_tensor(
                    kvbd[0:D, :, 0:D], stg2ab[0:D, 0, :, 0:D],
                    stg2ab[0:D, 1, :, 0:D], Add,
                )
                nc.vector.tensor_tensor(
                    kvbd[D:P, :, D:P], stg2ab[D:P, 0, :, D:P],
                    stg2ab[D:P, 1, :, D:P], Add,
                )
                nc.vector.tensor_tensor(
                    ksf[0:D, 0 : NCT * H : H + 2], stg2ab[0:D, 0, :, P],
                    stg2ab[0:D, 1, :, P], Add,
                )
                nc.vector.tensor_tensor(
                    ksf[D:P, 1 : NCT * H : H + 2], stg2ab[D:P, 0, :, P],
                    stg2ab[D:P, 1, :, P], Add,
                )
            else:
                nc.sync.dma_start(
                    stg2, ccin.rearrange("p (a v) -> p a v", a=NCT)
                )
                nc.vector.tensor_copy(kvbd[0:D, :, 0:D], stg2[0:D, :, 0:D])
                nc.vector.tensor_copy(kvbd[D:P, :, D:P], stg2[D:P, :, D:P])
                nc.vector.tensor_copy(
                    ksf[0:D, 0 : NCT * H : H + 2], stg2[0:D, :, P]
                )
                nc.vector.tensor_copy(
                    ksf[D:P, 1 : NCT * H : H + 2], stg2[D:P, :, P]
                )

            for lc in range(4):
                # denominators for the whole 512-row chunk in one psum bank
                # (one eps-add + one reciprocal instead of four)
                pd = dps.tile([P, 4, H], F32, tag="pd")
                for lt in range(4):
                    lsl = slice(lt * P, (lt + 1) * P)
                    for ct in range(NCT):
                        mm(pd[:, lt, :], qTs[lc][:, ct, lsl],
                           ksbd[:, ct, :],
                           start=(lt == 0 and ct == 0),
                           stop=(lt == 3 and ct == NCT - 1))
                zl = p2.tile([P, 4, H], F32, tag="zl")
                nc.scalar.activation(zl, pd, Copy, bias=EPS)
                nc.vector.reciprocal(zl, zl)
                for lt in range(4):
                    lsl = slice(lt * P, (lt + 1) * P)
                    po = pops.tile([P, NCT, P], F32, tag="po")
                    for hp in range(NCT):
                        mm(po[:, hp, :], qTs[lc][:, hp, lsl],
                           kvbd[:, hp, :],
                           start=(hp == 0), stop=(hp == NCT - 1))
                    # out = po * z broadcast: ONE DVE op so po's psum bank
                    # is freed by a single reader
                    osb = osbp.tile([P, C], BF, tag="osb")
                    nc.vector.tensor_tensor(
                        osb.rearrange("p (h v) -> p h v", h=H),
                        po.rearrange("p a (b v) -> p (a b) v", b=2),
                        zl[:, lt, :, None].to_broadcast([P, H, D]),
                        Mult,
                    )
                    nc.sync.dma_start(
                        outb[(lc * 4 + lt) * P : (lc * 4 + lt + 1) * P, :], osb
                    )


_CACHE = {}


def _get_nc(reps=1, with_bias=False):
    key = ("nc", reps, with_bias, SPLIT_S, SKIP_CC)
    if key not in _CACHE:
        _CACHE[key] = _build_nc(reps, with_bias, SPLIT_S)
    return _CACHE[key]


def _make_runner(nc):
    """Build a reusable jitted SPMD runner for `nc` (mirrors
    bass2jax.run_bass_via_pjrt's multi-core branch, but caches the jit so
    repeated calls don't re-lower/re-compile)."""
    import jax
    from jax.sharding import Mesh, PartitionSpec
    from jax.experimental.shard_map import shard_map

    import concourse.mybir as mb
    from concourse import bass2jax

    bass2jax.install_neuronx_cc_hook()

    n_cores = 8
    partition_name = (
        nc.partition_id_tensor.name if nc.partition_id_tensor else None
    )
    in_names, out_names, out_avals, zero_shapes = [], [], [], []
    for alloc in nc.m.functions[0].allocations:
        if not isinstance(alloc, mb.MemoryLocationSet):
            continue
        name = alloc.memorylocations[0].name
        if alloc.kind == "ExternalInput":
            if name != partition_name:
                in_names.append(name)
        elif alloc.kind == "ExternalOutput":
            shape = tuple(alloc.tensor_shape)
            dtype = mb.dt.np(alloc.dtype)
            out_names.append(name)
            out_avals.append(jax.core.ShapedArray(shape, dtype))
            zero_shapes.append((shape, dtype))
    n_params = len(in_names)
    n_outs = len(out_names)
    all_names = in_names + out_names
    if partition_name is not None:
        all_names.append(partition_name)
    donate = tuple(range(n_params, n_params + n_outs))

    def _body(*args):
        operands = list(args)
        if partition_name is not None:
            operands.append(bass2jax.partition_id_tensor())
        outs = bass2jax._bass_exec_p.bind(
            *operands,
            out_avals=tuple(out_avals),
            in_names=tuple(all_names),
            out_names=tuple(out_names),
            lowering_input_output_aliases=(),
            sim_require_finite=True,
            sim_require_nnan=True,
            nc=nc,
        )
        return tuple(outs)

    devices = jax.devices()[:n_cores]
    mesh = Mesh(np.asarray(devices), ("core",))
    in_specs = (PartitionSpec("core"),) * (n_params + n_outs)
    out_specs = (PartitionSpec("core"),) * n_outs
    sharded = jax.jit(
        shard_map(
            _body, mesh=mesh, in_specs=in_specs, out_specs=out_specs,
            check_rep=False,
        ),
        donate_argnums=donate,
        keep_unused=True,
    )

    def _zeros():
        return [
            np.zeros((n_cores * sh[0], *sh[1:]), dt) for sh, dt in zero_shapes
        ]

    def runner(concat_in):
        out_arrs = sharded(*concat_in, *_zeros())
        return [
            {
                name: np.asarray(out_arrs[i]).reshape(
                    n_cores, *out_avals[i].shape
                )[c]
                for i, name in enumerate(out_names)
            }
            for c in range(n_cores)
        ]

    def concat(maps):
        return [
            np.concatenate([np.asarray(m[name]) for m in maps], axis=0)
            for name in in_names
        ]

    def timed(concat_in, n=10, warmup=2):
        """Time `n` executions with device-resident inputs and on-device
        donated zero outputs, so per-call host traffic is ~zero."""
        import time as _time
        import jax.numpy as jnp
        from jax.sharding import NamedSharding

        sh = NamedSharding(mesh, PartitionSpec("core"))
        dev_in = [jax.device_put(a, sh) for a in concat_in]

        @jax.jit
        def _mkzeros():
            return tuple(
                jnp.zeros((n_cores * s[0], *s[1:]), d) for s, d in zero_shapes
            )

        _mkzeros = jax.jit(_mkzeros, out_shardings=(sh,) * n_outs)
        times = []
        for i in range(warmup + n):
            z = jax.block_until_ready(_mkzeros())
            t0 = _time.perf_counter()
            outs = sharded(*dev_in, *z)
            jax.block_until_ready(outs)
            dt = _time.perf_counter() - t0
            if i >= warmup:
                times.append(dt)
        return times

    return runner, concat, timed


def _in_maps(x, guidance, Wq, bq, Wk, bk, Wv, bv):
    x = np.asarray(x, dtype=np.float32).astype(NBF)
    guidance = np.asarray(guidance, dtype=np.float32).astype(NBF)
    wqt = np.ascontiguousarray(np.asarray(Wq, dtype=np.float32).T).astype(NBF)
    wkt = np.ascontiguousarray(np.asarray(Wk, dtype=np.float32).T).astype(NBF)
    wvt = np.ascontiguousarray(np.asarray(Wv, dtype=np.float32).T).astype(NBF)
    bq = np.ascontiguousarray(bq, dtype=np.float32)
    bk = np.asarray(bk, dtype=np.float32).astype(NBF)
    bv = np.asarray(bv, dtype=np.float32).astype(NBF)
    maps = []
    for core in range(8):
        b, half = core // 2, core % 2
        xb_c = (
            x[b, half * (S // 2) : (half + 1) * (S // 2)] if SPLIT_S else x[b]
        )
        maps.append(
            {
                "xb": np.ascontiguousarray(xb_c),
                "gb": np.ascontiguousarray(
                    guidance[b, half * LC : (half + 1) * LC]
                ),
                "wqt": wqt,
                "wkt": wkt,
                "wvt": wvt,
                "bq": bq,
                "bk": bk,
                "bv": bv,
            }
        )
    return maps


def _gather(results):
    B = 4
    out = np.empty((B, 2 * LC, C), dtype=np.float32)
    for core in range(8):
        b, half = core // 2, core % 2
        out[b, half * LC : (half + 1) * LC] = results[core]["outb"].astype(
            np.float32
        )
    return out


def run(inputs, reps=1):
    with_bias = bool(
        np.any(inputs["bq"]) or np.any(inputs["bk"]) or np.any(inputs["bv"])
    )
    nc = _get_nc(reps, with_bias)
    key = ("runner", reps, with_bias, SPLIT_S, SKIP_CC)
    if key not in _CACHE:
        _CACHE[key] = _make_runner(nc)
    runner, concat, timed = _CACHE[key]
    maps = _in_maps(**inputs)
    return runner, timed, concat(maps)


def kernel(**inputs):
    runner, _, concat_in = run(inputs)
    return _gather(runner(concat_in))
